# revision 5
# baseline (speedup 1.0000x reference)
"""ContrastiveTokenLoss on Trainium2 (8 NeuronCores, Bass/Tile).

Problem (hardcoded): input [2, 2048, 32000] f32 logits, target [2, 2048] int.
ct_len = round(2048*0.25) = 512, win = round(512*0.5) = 256,
IGNORE_INDEX = -100, PAD_ID = 0.

loss = sum_{b, i<512} valid(b,i) * log1p( sum_{j in [i-256, i), tgt[b,j]!=0}
           exp(x[b,i,tgt[b,j]] - x[b,i,tgt_safe[b,i]]) ) / max(#valid, 1)

Sharding: core c = 4*b + k owns 128 contiguous positions [128k, 128k+128) of
batch b (single-batch slabs keep the window overhead at one 256-token tail
instead of two).  Each core gets its logits slab vocab-major ([32001, 128]:
the one vocab row the loss needs per window token is contiguous; row 32000
is a -1e9 sentinel for PAD / out-of-range window tokens), gather row indices,
and one constant table (two pre-transposed additive band-mask blocks | 128x128
identity, identical for every core).

The window of 128 positions spans at most 3 aligned 128-token chunks:
  slot0 = own-token chunk  (band mask D keeps j <= i; the diagonal entry is
          the positive logit, and its exp(0)=1 doubles as log1p's "+1")
  slot1 = previous chunk   (always fully inside the 256-window: no mask)
  slot2 = chunk before that (band-start mask S keeps j >= i)
Cores with k < 2 redirect the missing slots to the sentinel row, which
zeroes their exp contribution; the program is identical on all 8 cores.

On-device per core: 3 (4 when PAD/ignore tokens are present) indirect DMAs
gather the window rows (512 B contiguous each, ~192 KB instead of the 16 MB
slab), PE preloads the mask blocks into PSUM on the transpose datapath and
then transpose-ACCUMULATES each gathered chunk on top, DVE extracts the
positive logit from the slot0 diagonal, ACT computes fused exp+row-sum with
-pos as a per-partition bias, and the [128, 3] per-(position, chunk) exp
sums DMA straight out.  The host finishes with log/log1p + masked mean over
the 1024 positions (trivial), avoiding the Exp->Ln ACT-table swap and the
very-slow gpsimd cross-partition reduce entirely.
"""

import numpy as np
from contextlib import ExitStack

import concourse.bass as bass
import concourse.bacc as bacc
import concourse.mybir as mybir
import concourse.tile as tile
from concourse.bass_utils import run_bass_kernel_spmd

B, T, V = 2, 2048, 32000
CT = 512
WIN = 256
IGNORE_INDEX = -100
PAD_ID = 0
NCORES = 8
P = 128                    # positions per core == partition rows
NWIN = 3                   # window chunks per core
F32 = mybir.dt.float32
I32 = mybir.dt.int32

_CACHE = {}


def _build(pos_chunk):
    """pos_chunk=False: 3 gathers, pos read off the slot0 diagonal (exact
    when every target in [0, CT) is > 0, which the host checks).
    pos_chunk=True: an extra leading gather chunk holds the own-target rows
    unsentineled so PAD own-targets still produce the right pos."""
    nch = NWIN + 1 if pos_chunk else NWIN
    nc = bacc.Bacc("TRN2", target_bir_lowering=False)
    xt = nc.dram_tensor("xt", [V + 1, P], F32, kind="ExternalInput")
    idx = nc.dram_tensor("idx", [P, nch], I32, kind="ExternalInput")
    cst = nc.dram_tensor("cst", [P, 3 * P], F32, kind="ExternalInput")
    out = nc.dram_tensor("out", [P, NWIN], F32, kind="ExternalOutput")

    with ExitStack() as ctx:
        tc = ctx.enter_context(tile.TileContext(nc))
        sb = ctx.enter_context(tc.tile_pool(name="sb", bufs=1))
        ps = ctx.enter_context(tc.tile_pool(name="ps", bufs=1, space="PSUM"))

        # gather row indices first so the gathers start as early as possible
        it = sb.tile([P, nch], I32)
        nc.sync.dma_start(it[:], idx[:])

        cst_sb = sb.tile([P, 3 * P], F32)
        nc.sync.dma_start(cst_sb[:], cst[:])
        ident = cst_sb[:, 2 * P : 3 * P]

        # fire the gathers (Pool engine SWDGE; they pipeline head-to-tail)
        gts = []
        for c in range(nch):
            gt = sb.tile([P, P], F32, tag=f"gt{c}")
            nc.gpsimd.indirect_dma_start(
                out=gt[:],
                out_offset=None,
                in_=xt[:],
                in_offset=bass.IndirectOffsetOnAxis(ap=it[:, c : c + 1], axis=0),
            )
            gts.append(gt)

        # Preload the additive band masks into slot 0/2's PSUM banks
        # (pt = stored_block.T via the transpose datapath) before the gathers
        # land; the chunk transpose then ACCUMULATES gt.T on top, so exp can
        # read (gt.T + mask) straight from PSUM with no DVE add.  Slot 1 is
        # always fully inside the window: no mask, plain transpose.
        pts = []
        for w in range(NWIN):
            pt = ps.tile([P, P], F32, tag=f"pt{w}", space="PSUM")
            pts.append(pt)
        nc.tensor.matmul(
            out=pts[0][:], lhsT=cst_sb[:, 0:P], rhs=ident,
            is_transpose=True, start=True, stop=False,
        )
        nc.tensor.matmul(
            out=pts[2][:], lhsT=cst_sb[:, P : 2 * P], rhs=ident,
            is_transpose=True, start=True, stop=False,
        )

        pd = sb.tile([P, P], F32)
        npos = sb.tile([P, 1], F32)
        r5 = sb.tile([P, NWIN], F32)
        nc.vector.memset(r5[:], 0.0)
        e = sb.tile([P, P], F32)

        if pos_chunk:
            # chunk 0 rows are the own targets: pos[p] = gt.T[p, p]
            ptp = ps.tile([P, P], F32, tag="ptp", space="PSUM")
            nc.tensor.transpose(out=ptp[:], in_=gts[0][:], identity=ident)
            nc.vector.tensor_tensor(pd[:], ptp[:], ident, mybir.AluOpType.mult)
            nc.vector.reduce_sum(
                npos[:], pd[:], axis=mybir.AxisListType.X, negate=True
            )

        off = 1 if pos_chunk else 0
        for w in range(NWIN):
            g = gts[w + off]
            nc.tensor.matmul(
                out=pts[w][:], lhsT=g[:], rhs=ident,
                is_transpose=True, start=(w == 1), stop=True,
            )
            if w == 0 and not pos_chunk:
                # mask diag is 0, so psum diag == gt.T diag == pos
                nc.vector.tensor_tensor(
                    pd[:], pts[0][:], ident, mybir.AluOpType.mult
                )
                nc.vector.reduce_sum(
                    npos[:], pd[:], axis=mybir.AxisListType.X, negate=True
                )
            # fused exp(chunk - pos) with per-row accumulation, from PSUM
            nc.scalar.activation(
                e[:], pts[w][:], mybir.ActivationFunctionType.Exp,
                bias=npos[:], scale=1.0, accum_out=r5[:, w : w + 1],
            )
        nc.sync.dma_start(out[:], r5[:])
    nc.compile()
    return nc


def _get_nc(pos_chunk):
    key = f"nc{pos_chunk}"
    if key not in _CACHE:
        _CACHE[key] = _build(pos_chunk)
    return _CACHE[key]


def _consts(pos_chunk):
    key = f"cst{pos_chunk}"
    if key not in _CACHE:
        il = np.arange(P)[:, None]
        jj = np.arange(P)[None, :]
        # slot0 mask D: keep j <= i (diagonal kept at 0 in the fast variant,
        # where its exp term is exactly 1 and plays log1p's "+1"); the slow
        # variant masks the diagonal too (own token is never its own negative)
        if pos_chunk:
            Dm = np.where(jj < il, 0.0, -1e9)
        else:
            Dm = np.where(jj <= il, 0.0, -1e9)
        # slot2 mask S: keep j >= i (distance <= 256)
        Sm = np.where(jj >= il, 0.0, -1e9)
        # stored pre-transposed: the PSUM preload runs on the transpose
        # datapath, so pt = stored.T
        cstv = np.concatenate(
            [Dm.T, Sm.T, np.eye(P)], axis=1
        ).astype(np.float32)
        _CACHE[key] = np.ascontiguousarray(cstv)
    return _CACHE[key]


def kernel(input, target, _trace=False):
    input = np.asarray(input, dtype=np.float32)
    target = np.asarray(target)
    t32 = target[:, :CT].astype(np.int32)  # [2, 512]

    # fast path: pos can be read off the slot0 diagonal iff no target in the
    # contrastive range is PAD (0) or negative
    pos_chunk = bool((t32 <= 0).any())
    cstv = _consts(pos_chunk)

    in_maps = []
    for core in range(NCORES):
        b, k = divmod(core, 4)
        s = P * k
        cols = []
        if pos_chunk:
            cols.append(np.maximum(t32[b, s : s + P], 0))
        for j in range(NWIN):  # slot j holds tokens [s - 128j, s - 128j + 128)
            lo = s - P * j
            if lo < 0:
                cols.append(np.full(P, V, np.int32))  # unused slot: sentinel
            elif pos_chunk:
                w = t32[b, lo : lo + P]
                cols.append(np.where(w == PAD_ID, V, np.maximum(w, 0)))
            else:
                cols.append(t32[b, lo : lo + P])  # all in [1, V)
        idxs = np.ascontiguousarray(np.stack(cols, axis=1).astype(np.int32))
        xtk = np.empty((V + 1, P), np.float32)
        xtk[:V] = input[b, s : s + P, :].T
        xtk[V:] = -1e9
        in_maps.append({"xt": xtk, "idx": idxs, "cst": cstv})

    nc = _get_nc(pos_chunk)
    br = run_bass_kernel_spmd(
        nc, in_maps, core_ids=list(range(NCORES)), trace=_trace,
        **({"trace_cores": list(range(NCORES))} if _trace else {}),
    )
    r = np.stack([res["out"] for res in br.results])  # [8, 128, 3]
    rsum = r.astype(np.float64).sum(axis=2)           # [8, 128]
    if pos_chunk:
        valid = (t32 != IGNORE_INDEX).reshape(2, 4, P).reshape(NCORES, P)
        losses = np.where(valid, np.log1p(rsum), 0.0)
        denom = max(int(valid.sum()), 1)
    else:
        losses = np.log(rsum)  # diagonal exp(0)=1 supplies log1p's "+1"
        denom = rsum.size
    kernel.last_results = br
    return np.asarray(np.float32(losses.sum() / denom))
